# revision 16
# baseline (speedup 1.0000x reference)
"""MoE (16 experts, top-2, SwiGLU D=1024 H=2816, S=4096 tokens) on 8 trn2 cores.

Strategy (expert-parallel, per the sharding hint):
  - Host: fp32 router (gate matmul + softmax + top-2 + weight norm), stable
    sort of token replicas by expert, bucketize. 8 largest buckets -> slot 0
    of cores 0..7, 8 smallest -> slot 1 (minimizes padded capacities C0/C1).
  - Device (SPMD, one program on all 8 cores): each core runs the SwiGLU MLP
    for its 2 experts over capacity-padded token buckets, entirely in bf16
    matmuls with fp32 PSUM accumulation.
  - Host: apply routing weights, concatenate buckets, invert the sort, sum
    the K=2 replicas.

Device layouts (T = transposed so the contraction dim is the partition dim):
  xs[s]  [KT,128,C]  = x_bucket.T              (bf16)   KT = D/128 = 8
  wg/wu  [KT,128,H]  = w_gate[e].T / w_up[e].T (bf16)
  wd     [HT,128,D]  = w_down[e].T             (bf16)   HT = H/128 = 22
  y      [DT,128,C]  = y_bucket.T              (f32)    DT = D/128 = 8

Phase 1 (per expert, per h-tile): hT[h,c] = silu(gT)*uT, gT = wgT.T @ xsT.
Phase 2 (per d-tile): yT[d,c] = sum_h wdT[h,d-tile].T @ hT[h,c].
Both phases stream token columns; weights are the stationary operand.
"""

import sys

import numpy as np

if "/opt/trn_rl_repo" not in sys.path:
    sys.path.insert(0, "/opt/trn_rl_repo")

import ml_dtypes

E, TOPK, D, H, S = 16, 2, 1024, 2816, 4096
P = 128
KT = D // P   # 8 k-tiles (phase-1 contraction)
HT = H // P   # 22 h-tiles (phase-2 contraction)
DT = D // P   # 8 d-tiles (phase-2 output rows)
NCORES = 8
BF16 = ml_dtypes.bfloat16

_PROGRAM_CACHE = {}


def _pad64(x):
    return max(64, ((int(x) + 63) // 64) * 64)


def _chunks(C):
    """Split C (multiple of 64) into <=512-sized chunks, each a multiple of 64."""
    n = -(-C // 512)
    p64 = C // 64
    sizes = [(p64 // n + (1 if i < p64 % n else 0)) * 64 for i in range(n)]
    out, off = [], 0
    for sz in sizes:
        out.append((off, sz))
        off += sz
    return out


def _build_program(C0, C1, warmup_mms=0):
    from contextlib import ExitStack

    import concourse.mybir as mybir
    import concourse.tile as tile
    from concourse import bacc

    dt = mybir.dt
    AF = mybir.ActivationFunctionType

    nc = bacc.Bacc(None, target_bir_lowering=False, debug=False)

    ios = []
    for s, C in ((0, C0), (1, C1)):
        xs = nc.dram_tensor(f"xs{s}", [KT, P, C], dt.bfloat16, kind="ExternalInput")
        wg = nc.dram_tensor(f"wg{s}", [KT, P, H], dt.bfloat16, kind="ExternalInput")
        wu = nc.dram_tensor(f"wu{s}", [KT, P, H], dt.bfloat16, kind="ExternalInput")
        wd = nc.dram_tensor(f"wd{s}", [HT, P, D], dt.bfloat16, kind="ExternalInput")
        y = nc.dram_tensor(f"y{s}", [DT, P, C], dt.float32, kind="ExternalOutput")
        ios.append((C, xs, wg, wu, wd, y))

    WCH = 512  # weight h-column chunk per DMA

    with tile.TileContext(nc) as tc, ExitStack() as ctx:
        wpool = ctx.enter_context(tc.tile_pool(name="w", bufs=3))
        wdpool = ctx.enter_context(tc.tile_pool(name="wd", bufs=1))
        xpool = ctx.enter_context(tc.tile_pool(name="xs", bufs=2))
        hpool = ctx.enter_context(tc.tile_pool(name="hT", bufs=2))
        spool = ctx.enter_context(tc.tile_pool(name="tmp", bufs=4))
        ypool = ctx.enter_context(tc.tile_pool(name="y", bufs=3))
        pspool = ctx.enter_context(tc.tile_pool(name="ps", bufs=2, space="PSUM"))
        pypool = ctx.enter_context(tc.tile_pool(name="py", bufs=2, space="PSUM"))

        # PE warmup matmuls are DISABLED (warmup_mms=0): a synchronized dense
        # matmul burst across all 8 cores right at kernel start trips the P0
        # power-state downclock (PE 2.4 -> 2.0 GHz for the entire run, +20%
        # measured). The ~2us HAM cold-start penalty is far cheaper.
        if warmup_mms:
            wz = spool.tile([P, 512], dt.bfloat16, tag="warm")
            nc.vector.memset(wz[:], 0.0)
            pw = pspool.tile([P, 512], dt.float32, tag="pg")
            for i in range(warmup_mms):
                nc.tensor.matmul(
                    pw[:], wz[:, :P], wz[:], start=(i == 0), stop=(i == warmup_mms - 1)
                )

        for C, xs_d, wg_d, wu_d, wd_d, y_d in ios:
            chunks = _chunks(C)
            cmax = max(cw for _, cw in chunks)

            xs_sb = xpool.tile([P, KT, C], dt.bfloat16, tag="xs")
            nc.sync.dma_start(
                out=xs_sb[:, : KT // 2],
                in_=xs_d[: KT // 2].rearrange("k p c -> p k c"),
            )
            nc.sync.dma_start(
                out=xs_sb[:, KT // 2 :],
                in_=xs_d[KT // 2 :].rearrange("k p c -> p k c"),
            )
            hT = hpool.tile([P, HT, C], dt.bfloat16, tag="hT")

            # ---- phase 1: hT = silu(wgT.T @ xsT) * (wuT.T @ xsT), per h-tile
            # First h-chunk is small so the first matmul chain starts sooner.
            h_chunks = []
            hc = 0
            for sz in (256, 256) + (WCH,) * 8:
                if hc >= H:
                    break
                sz = min(sz, H - hc)
                h_chunks.append((hc, sz))
                hc += sz
            assert sum(sz for _, sz in h_chunks) == H
            for hc, hw_ in h_chunks:
                wg_sb = wpool.tile([P, KT, hw_], dt.bfloat16, tag="wg")
                nc.sync.dma_start(
                    out=wg_sb[:],
                    in_=wg_d[:, :, hc : hc + hw_].rearrange("k p h -> p k h"),
                )
                wu_sb = wpool.tile([P, KT, hw_], dt.bfloat16, tag="wu")
                nc.sync.dma_start(
                    out=wu_sb[:],
                    in_=wu_d[:, :, hc : hc + hw_].rearrange("k p h -> p k h"),
                )
                for hi in range(hw_ // P):
                    h = hc // P + hi
                    for c0, cw in chunks:
                        pg = pspool.tile([P, cmax], dt.float32, tag="pg")
                        pu = pspool.tile([P, cmax], dt.float32, tag="pu")
                        for k in range(KT):
                            nc.tensor.matmul(
                                pg[:, :cw],
                                wg_sb[:, k, hi * P : (hi + 1) * P],
                                xs_sb[:, k, c0 : c0 + cw],
                                start=(k == 0),
                                stop=(k == KT - 1),
                            )
                        for k in range(KT):
                            nc.tensor.matmul(
                                pu[:, :cw],
                                wu_sb[:, k, hi * P : (hi + 1) * P],
                                xs_sb[:, k, c0 : c0 + cw],
                                start=(k == 0),
                                stop=(k == KT - 1),
                            )
                        sg = spool.tile([P, cmax], dt.float32, tag="sg")
                        nc.scalar.activation(
                            out=sg[:, :cw], in_=pg[:, :cw], func=AF.Sigmoid
                        )
                        nc.vector.tensor_mul(
                            out=sg[:, :cw], in0=sg[:, :cw], in1=pg[:, :cw]
                        )
                        nc.vector.tensor_mul(
                            out=hT[:, h, c0 : c0 + cw], in0=sg[:, :cw], in1=pu[:, :cw]
                        )

            # wd arrives during phase 1 (issued after phase-1 weight DMAs so it
            # doesn't steal bandwidth from the critical path at kernel start)
            wd_sb = wdpool.tile([P, HT, D], dt.bfloat16, tag="wd")
            nc.sync.dma_start(out=wd_sb[:], in_=wd_d.rearrange("h p d -> p h d"))

            # ---- phase 2: yT[d-tile] = sum_h wdT[h, d-tile].T @ hT[h]
            for dtile in range(DT):
                y_sb = ypool.tile([P, C], dt.float32, tag="y")
                for c0, cw in chunks:
                    py = pypool.tile([P, cmax], dt.float32, tag="py")
                    for h in range(HT):
                        nc.tensor.matmul(
                            py[:, :cw],
                            wd_sb[:, h, dtile * P : (dtile + 1) * P],
                            hT[:, h, c0 : c0 + cw],
                            start=(h == 0),
                            stop=(h == HT - 1),
                        )
                    nc.scalar.activation(
                        out=y_sb[:, c0 : c0 + cw], in_=py[:, :cw], func=AF.Copy
                    )
                nc.sync.dma_start(out=y_d[dtile], in_=y_sb[:])

    nc.compile()
    return nc


def _get_program(C0, C1):
    key = (C0, C1)
    if key not in _PROGRAM_CACHE:
        _PROGRAM_CACHE[key] = _build_program(C0, C1)
    return _PROGRAM_CACHE[key]


def _router_host(hidden_states, gate_w):
    hs = np.asarray(hidden_states, np.float32)
    logits = hs @ np.asarray(gate_w, np.float32).T  # [S, E]
    m = logits.max(axis=1, keepdims=True)
    ex = np.exp(logits - m)
    probs = ex / ex.sum(axis=1, keepdims=True)
    top_i = np.argsort(-probs, axis=1, kind="stable")[:, :TOPK].astype(np.int32)
    tw = np.take_along_axis(probs, top_i, axis=1)
    rw = (tw / tw.sum(axis=1, keepdims=True)).astype(np.float32)
    return logits, top_i, rw


def kernel_full(hidden_states, gate_w, w_gate, w_up, w_down, trace=False):
    from concourse.bass_utils import run_bass_kernel_spmd

    hs = np.asarray(hidden_states, np.float32)
    S_n = hs.shape[0]
    logits, top_i, rw = _router_host(hs, gate_w)

    # ---- dispatch bookkeeping (host) ----
    flat_e = top_i.reshape(-1)
    sort_idx = np.argsort(flat_e, kind="stable")
    sizes = np.bincount(flat_e, minlength=E)
    starts = np.concatenate([[0], np.cumsum(sizes)])
    tok_sorted = sort_idx // TOPK  # original token of each sorted replica
    rw_sorted = rw.reshape(-1)[sort_idx]

    order = np.argsort(-sizes, kind="stable")  # experts by bucket size desc
    slot_experts = [order[:NCORES], order[NCORES:]]  # slot0 = 8 biggest
    C0 = _pad64(sizes[slot_experts[0]].max())
    C1 = _pad64(sizes[slot_experts[1]].max())

    nc = _get_program(C0, C1)

    hs_bf = hs.astype(BF16)
    in_maps = []
    for c in range(NCORES):
        m = {}
        for s, C in ((0, C0), (1, C1)):
            e = int(slot_experts[s][c])
            n = int(sizes[e])
            toks = tok_sorted[starts[e] : starts[e] + n]

            xsT = np.zeros((D, C), dtype=BF16)
            xsT[:, :n] = hs_bf[toks].T
            m[f"xs{s}"] = np.ascontiguousarray(xsT.reshape(KT, P, C))

            m[f"wg{s}"] = np.ascontiguousarray(
                np.asarray(w_gate[e], np.float32).T.astype(BF16).reshape(KT, P, H)
            )
            m[f"wu{s}"] = np.ascontiguousarray(
                np.asarray(w_up[e], np.float32).T.astype(BF16).reshape(KT, P, H)
            )
            m[f"wd{s}"] = np.ascontiguousarray(
                np.asarray(w_down[e], np.float32).T.astype(BF16).reshape(HT, P, D)
            )
        in_maps.append(m)

    res = run_bass_kernel_spmd(nc, in_maps, list(range(NCORES)), trace=trace)

    # ---- combine (host): scale by routing weight, unsort, sum K replicas ----
    y_sorted = np.empty((S_n * TOPK, D), np.float32)
    for c in range(NCORES):
        for s, C in ((0, C0), (1, C1)):
            e = int(slot_experts[s][c])
            n = int(sizes[e])
            yT = res.results[c][f"y{s}"].reshape(D, C)  # [D, C]
            y_sorted[starts[e] : starts[e] + n] = yT[:, :n].T
    y_sorted *= rw_sorted[:, None]

    inv_sort = np.argsort(sort_idx, kind="stable")
    out = y_sorted[inv_sort].reshape(S_n, TOPK, D).sum(axis=1)
    return (out, logits.astype(np.float32), top_i), res


def kernel(hidden_states, gate_w, w_gate, w_up, w_down):
    outs, _ = kernel_full(hidden_states, gate_w, w_gate, w_up, w_down, trace=False)
    return outs


# revision 20
# speedup vs baseline: 1.0245x; 1.0245x over previous
"""MoE (16 experts, top-2, SwiGLU D=1024 H=2816, S=4096 tokens) on 8 trn2 cores.

Strategy (expert-parallel, per the sharding hint):
  - Host: fp32 router (gate matmul + softmax + top-2 + weight norm), stable
    sort of token replicas by expert, bucketize. 8 largest buckets -> slot 0
    of cores 0..7, 8 smallest -> slot 1 (minimizes padded capacities C0/C1).
  - Device (SPMD, one program on all 8 cores): each core runs the SwiGLU MLP
    for its 2 experts over capacity-padded token buckets, entirely in bf16
    matmuls with fp32 PSUM accumulation.
  - Host: apply routing weights, concatenate buckets, invert the sort, sum
    the K=2 replicas.

Device layouts (T = transposed so the contraction dim is the partition dim):
  xs[s]  [KT,128,C]  = x_bucket.T              (bf16)   KT = D/128 = 8
  wg/wu  [KT,128,H]  = w_gate[e].T / w_up[e].T (bf16)
  wd     [HT,128,D]  = w_down[e].T             (bf16)   HT = H/128 = 22
  y      [DT,128,C]  = y_bucket.T              (f32)    DT = D/128 = 8

Phase 1 (per expert, per h-tile): hT[h,c] = silu(gT)*uT, gT = wgT.T @ xsT.
Phase 2 (per d-tile): yT[d,c] = sum_h wdT[h,d-tile].T @ hT[h,c].
Both phases stream token columns; weights are the stationary operand.
"""

import sys

import numpy as np

if "/opt/trn_rl_repo" not in sys.path:
    sys.path.insert(0, "/opt/trn_rl_repo")

import ml_dtypes

E, TOPK, D, H, S = 16, 2, 1024, 2816, 4096
P = 128
KT = D // P   # 8 k-tiles (phase-1 contraction)
HT = H // P   # 22 h-tiles (phase-2 contraction)
DT = D // P   # 8 d-tiles (phase-2 output rows)
NCORES = 8
BF16 = ml_dtypes.bfloat16

_PROGRAM_CACHE = {}


def _pad2(x):
    """Capacities need no tile alignment (phase 2 streams token columns);
    pad only to an even count so bf16 rows stay 4-byte aligned."""
    return max(2, ((int(x) + 1) // 2) * 2)


def _chunks(C):
    """Split C into evenly-sized chunks of at most 512 (one PSUM bank f32)."""
    n = -(-C // 512)
    base, rem = C // n, C % n
    out, off = [], 0
    for i in range(n):
        sz = base + (1 if i < rem else 0)
        out.append((off, sz))
        off += sz
    return out


def _build_program(C0, C1, warmup_mms=0):
    from contextlib import ExitStack

    import concourse.mybir as mybir
    import concourse.tile as tile
    from concourse import bacc

    dt = mybir.dt
    AF = mybir.ActivationFunctionType

    nc = bacc.Bacc(None, target_bir_lowering=False, debug=False)

    ios = []
    for s, C in ((0, C0), (1, C1)):
        xs = nc.dram_tensor(f"xs{s}", [KT, P, C], dt.bfloat16, kind="ExternalInput")
        wg = nc.dram_tensor(f"wg{s}", [KT, P, H], dt.bfloat16, kind="ExternalInput")
        wu = nc.dram_tensor(f"wu{s}", [KT, P, H], dt.bfloat16, kind="ExternalInput")
        wd = nc.dram_tensor(f"wd{s}", [HT, P, D], dt.bfloat16, kind="ExternalInput")
        y = nc.dram_tensor(f"y{s}", [DT, P, C], dt.float32, kind="ExternalOutput")
        ios.append((C, xs, wg, wu, wd, y))

    WCH = 512  # weight h-column chunk per DMA

    with tile.TileContext(nc) as tc, ExitStack() as ctx:
        wpool = ctx.enter_context(tc.tile_pool(name="w", bufs=3))
        wdpool = ctx.enter_context(tc.tile_pool(name="wd", bufs=1))
        xpool = ctx.enter_context(tc.tile_pool(name="xs", bufs=2))
        hpool = ctx.enter_context(tc.tile_pool(name="hT", bufs=2))
        spool = ctx.enter_context(tc.tile_pool(name="tmp", bufs=4))
        ypool = ctx.enter_context(tc.tile_pool(name="y", bufs=3))
        pspool = ctx.enter_context(tc.tile_pool(name="ps", bufs=2, space="PSUM"))
        pypool = ctx.enter_context(tc.tile_pool(name="py", bufs=2, space="PSUM"))

        # PE warmup matmuls are DISABLED (warmup_mms=0): a synchronized dense
        # matmul burst across all 8 cores right at kernel start trips the P0
        # power-state downclock (PE 2.4 -> 2.0 GHz for the entire run, +20%
        # measured). The ~2us HAM cold-start penalty is far cheaper.
        if warmup_mms:
            wz = spool.tile([P, 512], dt.bfloat16, tag="warm")
            nc.vector.memset(wz[:], 0.0)
            pw = pspool.tile([P, 512], dt.float32, tag="pg")
            for i in range(warmup_mms):
                nc.tensor.matmul(
                    pw[:], wz[:, :P], wz[:], start=(i == 0), stop=(i == warmup_mms - 1)
                )

        for C, xs_d, wg_d, wu_d, wd_d, y_d in ios:
            chunks = _chunks(C)
            cmax = max(cw for _, cw in chunks)

            xs_sb = xpool.tile([P, KT, C], dt.bfloat16, tag="xs")
            hT = hpool.tile([P, HT, C], dt.bfloat16, tag="hT")

            # ---- phase 1: hT = silu(wgT.T @ xsT) * (wuT.T @ xsT), per h-tile
            # First h-chunk is small so the first matmul chain starts sooner.
            h_chunks = []
            hc = 0
            for sz in (256, 256) + (WCH,) * 8:
                if hc >= H:
                    break
                sz = min(sz, H - hc)
                h_chunks.append((hc, sz))
                hc += sz
            assert sum(sz for _, sz in h_chunks) == H
            for hc, hw_ in h_chunks:
                wg_sb = wpool.tile([P, KT, hw_], dt.bfloat16, tag="wg")
                nc.sync.dma_start(
                    out=wg_sb[:],
                    in_=wg_d[:, :, hc : hc + hw_].rearrange("k p h -> p k h"),
                )
                if hc == 0:
                    # xs arrives by column-chunk, after the first (small) wg
                    # chunk: the first matmul chain needs only wg[0:256] plus
                    # the first xs chunk (~1.1MB) instead of all of xs.
                    for c0, cw in chunks:
                        nc.sync.dma_start(
                            out=xs_sb[:, :, c0 : c0 + cw],
                            in_=xs_d[:, :, c0 : c0 + cw].rearrange("k p c -> p k c"),
                        )
                wu_sb = wpool.tile([P, KT, hw_], dt.bfloat16, tag="wu")
                nc.sync.dma_start(
                    out=wu_sb[:],
                    in_=wu_d[:, :, hc : hc + hw_].rearrange("k p h -> p k h"),
                )
                for hi in range(hw_ // P):
                    h = hc // P + hi
                    for c0, cw in chunks:
                        pg = pspool.tile([P, cmax], dt.float32, tag="pg")
                        pu = pspool.tile([P, cmax], dt.float32, tag="pu")
                        for k in range(KT):
                            nc.tensor.matmul(
                                pg[:, :cw],
                                wg_sb[:, k, hi * P : (hi + 1) * P],
                                xs_sb[:, k, c0 : c0 + cw],
                                start=(k == 0),
                                stop=(k == KT - 1),
                            )
                        for k in range(KT):
                            nc.tensor.matmul(
                                pu[:, :cw],
                                wu_sb[:, k, hi * P : (hi + 1) * P],
                                xs_sb[:, k, c0 : c0 + cw],
                                start=(k == 0),
                                stop=(k == KT - 1),
                            )
                        sg = spool.tile([P, cmax], dt.float32, tag="sg")
                        nc.scalar.activation(
                            out=sg[:, :cw], in_=pg[:, :cw], func=AF.Sigmoid
                        )
                        nc.vector.tensor_mul(
                            out=sg[:, :cw], in0=sg[:, :cw], in1=pg[:, :cw]
                        )
                        nc.vector.tensor_mul(
                            out=hT[:, h, c0 : c0 + cw], in0=sg[:, :cw], in1=pu[:, :cw]
                        )

            # wd arrives during phase 1 (issued after phase-1 weight DMAs so it
            # doesn't steal bandwidth from the critical path at kernel start)
            wd_sb = wdpool.tile([P, HT, D], dt.bfloat16, tag="wd")
            nc.sync.dma_start(out=wd_sb[:], in_=wd_d.rearrange("h p d -> p h d"))

            # ---- phase 2: yT[d-tile] = sum_h wdT[h, d-tile].T @ hT[h]
            for dtile in range(DT):
                y_sb = ypool.tile([P, C], dt.float32, tag="y")
                for c0, cw in chunks:
                    py = pypool.tile([P, cmax], dt.float32, tag="py")
                    for h in range(HT):
                        nc.tensor.matmul(
                            py[:, :cw],
                            wd_sb[:, h, dtile * P : (dtile + 1) * P],
                            hT[:, h, c0 : c0 + cw],
                            start=(h == 0),
                            stop=(h == HT - 1),
                        )
                    nc.scalar.activation(
                        out=y_sb[:, c0 : c0 + cw], in_=py[:, :cw], func=AF.Copy
                    )
                nc.sync.dma_start(out=y_d[dtile], in_=y_sb[:])

    nc.compile()
    return nc


def _get_program(C0, C1):
    key = (C0, C1)
    if key not in _PROGRAM_CACHE:
        _PROGRAM_CACHE[key] = _build_program(C0, C1)
    return _PROGRAM_CACHE[key]


def _router_host(hidden_states, gate_w):
    hs = np.asarray(hidden_states, np.float32)
    logits = hs @ np.asarray(gate_w, np.float32).T  # [S, E]
    m = logits.max(axis=1, keepdims=True)
    ex = np.exp(logits - m)
    probs = ex / ex.sum(axis=1, keepdims=True)
    top_i = np.argsort(-probs, axis=1, kind="stable")[:, :TOPK].astype(np.int32)
    tw = np.take_along_axis(probs, top_i, axis=1)
    rw = (tw / tw.sum(axis=1, keepdims=True)).astype(np.float32)
    return logits, top_i, rw


def kernel_full(hidden_states, gate_w, w_gate, w_up, w_down, trace=False):
    from concourse.bass_utils import run_bass_kernel_spmd

    hs = np.asarray(hidden_states, np.float32)
    S_n = hs.shape[0]
    logits, top_i, rw = _router_host(hs, gate_w)

    # ---- dispatch bookkeeping (host) ----
    flat_e = top_i.reshape(-1)
    sort_idx = np.argsort(flat_e, kind="stable")
    sizes = np.bincount(flat_e, minlength=E)
    starts = np.concatenate([[0], np.cumsum(sizes)])
    tok_sorted = sort_idx // TOPK  # original token of each sorted replica
    rw_sorted = rw.reshape(-1)[sort_idx]

    order = np.argsort(-sizes, kind="stable")  # experts by bucket size desc
    slot_experts = [order[:NCORES], order[NCORES:]]  # slot0 = 8 biggest
    C0 = _pad2(sizes[slot_experts[0]].max())
    C1 = _pad2(sizes[slot_experts[1]].max())

    nc = _get_program(C0, C1)

    hs_bf = hs.astype(BF16)
    in_maps = []
    for c in range(NCORES):
        m = {}
        for s, C in ((0, C0), (1, C1)):
            e = int(slot_experts[s][c])
            n = int(sizes[e])
            toks = tok_sorted[starts[e] : starts[e] + n]

            xsT = np.zeros((D, C), dtype=BF16)
            xsT[:, :n] = hs_bf[toks].T
            m[f"xs{s}"] = np.ascontiguousarray(xsT.reshape(KT, P, C))

            m[f"wg{s}"] = np.ascontiguousarray(
                np.asarray(w_gate[e], np.float32).T.astype(BF16).reshape(KT, P, H)
            )
            m[f"wu{s}"] = np.ascontiguousarray(
                np.asarray(w_up[e], np.float32).T.astype(BF16).reshape(KT, P, H)
            )
            m[f"wd{s}"] = np.ascontiguousarray(
                np.asarray(w_down[e], np.float32).T.astype(BF16).reshape(HT, P, D)
            )
        in_maps.append(m)

    res = run_bass_kernel_spmd(nc, in_maps, list(range(NCORES)), trace=trace)

    # ---- combine (host): scale by routing weight, unsort, sum K replicas ----
    y_sorted = np.empty((S_n * TOPK, D), np.float32)
    for c in range(NCORES):
        for s, C in ((0, C0), (1, C1)):
            e = int(slot_experts[s][c])
            n = int(sizes[e])
            yT = res.results[c][f"y{s}"].reshape(D, C)  # [D, C]
            y_sorted[starts[e] : starts[e] + n] = yT[:, :n].T
    y_sorted *= rw_sorted[:, None]

    inv_sort = np.argsort(sort_idx, kind="stable")
    out = y_sorted[inv_sort].reshape(S_n, TOPK, D).sum(axis=1)
    return (out, logits.astype(np.float32), top_i), res


def kernel(hidden_states, gate_w, w_gate, w_up, w_down):
    outs, _ = kernel_full(hidden_states, gate_w, w_gate, w_up, w_down, trace=False)
    return outs
